# revision 47
# baseline (speedup 1.0000x reference)
"""GAT (2-layer graph attention network) forward pass on 8 TRN2 NeuronCores.

Sharding: 1D node partition. Core c owns output rows I_c = [512c, 512c+512).
Each core holds the full (small) per-head projections Wh and computes the
attention row-block for its rows against all N columns, for all 8 heads,
entirely on-device. The only cross-core exchange is one AllGather of the
layer-2 projections Wh2 = h @ W_out (plus the t2 attention scalars shipped
as bf16 hi/lo pairs for fp32-equivalent precision).

Key algebraic trick (no transcendentals on the N x N tiles at all): with
z_ij = s1_i + s2_j, softmax row-normalization makes attention invariant to
any per-row positive scale, so

    exp(leakyrelu(z)) / e^{0.2 s1_i} = max(e^{0.8 s1_i} * e^{s2_j}, e^{0.2 s2_j})

With u = e^{0.8 s1}, b = e^{s2}, d = e^{0.2 s2} host-precomputed per node,
each N x N tile needs exactly one fused tensor_scalar (u*b max d, 4x-mode
bf16) and one mask tensor_tensor multiply (2x-mode), feeding the PE matmul
directly (j on partitions = contraction axis; row sums via a ones column).

j-chunks are processed in strided groups of 8 (jc = 4k + q); each group's
mask multiply is one batched tensor_tensor (6 chunks on DVE + 2 on GpSimd
to balance engine load).
"""

import numpy as np
import ml_dtypes

N = 4096
NFEAT = 512
NHID = 64
NHEADS = 8
NCLASS = 41
NCORES = 8
BLK = N // NCORES          # 512 rows per core
JC = N // 128              # 32 j-chunks
IC = BLK // 128            # 4 i-chunks of the owned block
GRP = 8                    # j-chunks per mask-multiply batch
NG = JC // GRP             # 4 groups (also 4 chunked collectives)

BF16 = ml_dtypes.bfloat16

_CACHE = {}

# Filled with the BassKernelResults of the most recent run (for test.py).
LAST_RESULTS = None


def _build_program():
    """Build + compile the Bass/Tile program once. Returns the Bacc object."""
    from contextlib import ExitStack
    import concourse.bass as bass
    import concourse.tile as tile
    import concourse.mybir as mybir
    from concourse import bacc
    from concourse.masks import make_identity

    f32 = mybir.dt.float32
    bf16 = mybir.dt.bfloat16
    EXP = mybir.ActivationFunctionType.Exp
    LN = mybir.ActivationFunctionType.Ln
    RELU = mybir.ActivationFunctionType.Relu
    MULT = mybir.AluOpType.mult
    ADD = mybir.AluOpType.add
    MAX = mybir.AluOpType.max
    SUB = mybir.AluOpType.subtract
    AX = mybir.AxisListType.X

    nc = bacc.Bacc("TRN2", target_bir_lowering=False, debug=False,
                   num_devices=NCORES)

    # ---- I/O ----
    whe = nc.dram_tensor("whe", [NG, 128, GRP, NHEADS * 65], bf16,
                         kind="ExternalInput").ap()
    adjt = nc.dram_tensor("adjt", [NG, 128, GRP, BLK], bf16,
                          kind="ExternalInput").ap()
    ublk = nc.dram_tensor("ublk", [NHEADS, BLK], bf16, kind="ExternalInput").ap()
    # sdb columns: [0:8] s2 per head, [8:16] d = e^{0.2 s2}, [16:24] b = e^{s2}
    sdb = nc.dram_tensor("sdb", [128, JC * 24], f32, kind="ExternalInput").ap()
    wout = nc.dram_tensor("wout", [NFEAT, NCLASS], bf16, kind="ExternalInput").ap()
    a1o = nc.dram_tensor("a1o", [NCLASS], f32, kind="ExternalInput").ap()
    a2o32 = nc.dram_tensor("a2o32", [JC * NCLASS], f32, kind="ExternalInput").ap()
    out = nc.dram_tensor("out", [BLK, NCLASS], f32, kind="ExternalOutput").ap()

    # internal DRAM: collective bounces + partition-broadcast bounces
    NCC = NCLASS + 3   # 41 Wh2 cols + t2 hi/lo + pad
    ccin = nc.dram_tensor("ccin", [BLK, NCC], bf16)
    ccout = nc.dram_tensor("ccout", [N, NCC], bf16)

    def dbcast(handle_or_ap, parts, count, offset=0):
        """AP reading `count` contiguous elements of a DRAM tensor, replicated
        across `parts` partitions (partition step 0)."""
        t = handle_or_ap.tensor if isinstance(handle_or_ap, bass.AP) else handle_or_ap
        base = handle_or_ap.offset if isinstance(handle_or_ap, bass.AP) else 0
        return bass.AP(tensor=t, offset=base + offset, ap=[[0, parts], [1, count]])

    with tile.TileContext(nc) as tc, ExitStack() as ctx:
        persist = ctx.enter_context(tc.tile_pool(name="persist", bufs=1))

        # ---------------- persistent tiles (DMA order = consumption order) ----
        # strided grouping: group q holds j-chunks jc = 4k + q, k = 0..7
        ubt = persist.tile([128, NHEADS, BLK], bf16, name="ubt", tag="ubt")
        nc.sync.dma_start(
            out=ubt, in_=bass.AP(tensor=ublk.tensor, offset=0,
                                 ap=[[0, 128], [BLK, NHEADS], [1, BLK]]))
        ub_t = [ubt[:, h, :] for h in range(NHEADS)]

        sS = persist.tile([128, JC, 24], f32, name="sS", tag="sS")
        nc.sync.dma_start(out=sS, in_=sdb.rearrange("p (c k) -> p c k", k=24))

        adj_g = []
        whe_g = []
        wheQ = whe.rearrange("q p k (h d) -> q p k h d", h=NHEADS)
        for g in range(NG):
            a = persist.tile([128, GRP, BLK], bf16, name=f"adjg{g}", tag=f"adjg{g}")
            nc.sync.dma_start(out=a, in_=adjt[g])
            adj_g.append(a)
            w = persist.tile([128, GRP, NHEADS, 65], bf16, name=f"wheg{g}",
                             tag=f"wheg{g}")
            nc.sync.dma_start(out=w, in_=wheQ[g])
            whe_g.append(w)

        woutB = persist.tile([128, IC, NCLASS], bf16, name="woutB", tag="woutB")
        nc.sync.dma_start(out=woutB, in_=wout.rearrange("(kc p) c -> p kc c", p=128))
        a1oB = persist.tile([128, NCLASS], f32, name="a1oB", tag="a1oB")
        nc.sync.dma_start(out=a1oB, in_=dbcast(a1o, 128, NCLASS))
        a2oB = persist.tile([128, GRP, NCLASS], f32, name="a2oB", tag="a2oB")
        nc.sync.dma_start(out=a2oB, in_=dbcast(a2o32, 128, GRP * NCLASS))

        ident = persist.tile([128, 128], f32, name="ident", tag="ident")
        make_identity(nc, ident)

        hT = persist.tile([128, IC, BLK], bf16, name="hT", tag="hT")

        # ---------------- pools ----------------
        tp = ctx.enter_context(tc.tile_pool(name="tp", bufs=4))
        ep = ctx.enter_context(tc.tile_pool(name="ep", bufs=2))
        ps1 = ctx.enter_context(tc.tile_pool(name="ps1", bufs=1, space="PSUM"))
        ps_small = ctx.enter_context(tc.tile_pool(name="pssm", bufs=1, space="PSUM"))
        dscratch = ctx.enter_context(tc.tile_pool(name="dscr", bufs=2, space="DRAM"))

        def jcof(g, k):
            return NG * k + g

        # ---------------- layer 1 ----------------
        # psW[q] accumulates Wh2 for the core's i-chunk q, fed incrementally
        # as each head pair's hT chunk completes.
        psW = [ps_small.tile([128, NCLASS], f32, tag=f"psW{q}", bufs=1,
                             name=f"psW{q}")
               for q in range(IC)]
        for h in range(NHEADS):
            psL = ps1.tile([65, BLK], f32, tag="psL", bufs=3)
            for g in range(NG):
                T2g = tp.tile([128, GRP, BLK], bf16, tag="T2g")
                for k in range(GRP):
                    jc = jcof(g, k)
                    # (u_i * b_j) max d_j  -- one fused 4x-mode DVE op
                    nc.vector.tensor_scalar(out=T2g[:, k, :], in0=ub_t[h],
                                            scalar1=sS[:, jc, 16 + h:17 + h],
                                            scalar2=sS[:, jc, 8 + h:9 + h],
                                            op0=MULT, op1=MAX)
                Pg = tp.tile([128, GRP, BLK], bf16, tag="Pg")
                nc.gpsimd.tensor_tensor(out=Pg[:, 6:, :], in0=T2g[:, 6:, :],
                                        in1=adj_g[g][:, 6:, :], op=MULT)
                nc.vector.tensor_tensor(out=Pg[:, :6, :], in0=T2g[:, :6, :],
                                        in1=adj_g[g][:, :6, :], op=MULT)
                for k in range(GRP):
                    nc.tensor.matmul(psL, lhsT=whe_g[g][:, k, h, :], rhs=Pg[:, k, :],
                                     start=(g == 0 and k == 0),
                                     stop=(g == NG - 1 and k == GRP - 1))

            # epilogue: normalize rows, apply ELU, write into hT (transposed h)
            rcp = ep.tile([32, BLK], f32, tag="rcp")
            nc.vector.reciprocal(out=rcp[0:1, :], in_=psL[64:65, :])
            rB = ep.tile([64, BLK], f32, tag="rB")
            if h == NHEADS - 1:
                # exposed tail: lowest-latency broadcast via quadrant shuffle
                # (partitions 1..31 of rcp are read but unused) + aligned copy
                nc.vector.stream_shuffle(out=rB[0:32, :], in_=rcp, mask=[0] * 32)
                nc.vector.tensor_copy(out=rB[32:64, :], in_=rB[0:32, :])
            else:
                # hidden chains: DRAM bounce costs no compute-engine time
                dden = dscratch.tile([BLK], f32, tag="dden")
                nc.sync.dma_start(out=dden, in_=rcp[0:1, :])
                nc.sync.dma_start(out=rB, in_=dbcast(dden, 64, BLK))
            ve = nc.vector if h == NHEADS - 1 else nc.gpsimd
            cp = ep.tile([64, BLK], f32, tag="cp")
            nc.scalar.copy(out=cp, in_=psL[:64, :])
            x1 = ep.tile([64, BLK], f32, tag="x1")
            ve.tensor_tensor(out=x1, in0=cp, in1=rB, op=MULT)
            # elu(x) = relu(x) - relu(1 - e^x)
            ex = ep.tile([64, BLK], f32, tag="ex")
            nc.scalar.activation(out=ex, in_=x1, func=EXP)
            exg = ep.tile([64, BLK], f32, tag="exg")
            nc.scalar.activation(out=exg, in_=ex, func=RELU, scale=-1.0, bias=1.0)
            r1 = ep.tile([64, BLK], f32, tag="r1")
            nc.scalar.activation(out=r1, in_=x1, func=RELU)
            ve.tensor_tensor(out=hT[64 * (h % 2):64 * (h % 2) + 64, h // 2, :],
                             in0=r1, in1=exg, op=SUB)
            if h % 2 == 1:
                kc = h // 2
                for q in range(IC):
                    nc.tensor.matmul(psW[q], lhsT=hT[:, kc, q * 128:(q + 1) * 128],
                                     rhs=woutB[:, kc, :],
                                     start=(kc == 0), stop=(kc == IC - 1))

        # ---------------- layer 2: Wh2 collect + single AllGather ----------------
        t1col = persist.tile([128, IC], f32, name="t1col", tag="t1col")
        wh2loc = persist.tile([128, IC, NCC], bf16, name="wh2loc", tag="wh2loc")
        nc.vector.memset(wh2loc[:, :, NCLASS + 2:], 0.0)
        for q in range(IC):
            nc.vector.tensor_copy(out=wh2loc[:, q, :NCLASS], in_=psW[q])
            tmp = ep.tile([128, NCLASS], f32, tag="tmp")
            nc.vector.tensor_tensor(out=tmp, in0=psW[q], in1=a1oB, op=MULT)
            nc.vector.tensor_reduce(out=t1col[:, q:q + 1], in_=tmp, axis=AX, op=ADD)
            # t2 local (fp32), shipped as bf16 hi + residual lo for full precision
            tmpb = ep.tile([128, NCLASS], f32, tag="tmpb")
            nc.vector.tensor_tensor(out=tmpb, in0=psW[q], in1=a2oB[:, 0, :], op=MULT)
            t2l = ep.tile([128, 1], f32, tag="t2l")
            nc.vector.tensor_reduce(out=t2l, in_=tmpb, axis=AX, op=ADD)
            nc.vector.tensor_copy(out=wh2loc[:, q, NCLASS:NCLASS + 1], in_=t2l)
            t2hi = ep.tile([128, 1], f32, tag="t2hi")
            nc.vector.tensor_copy(out=t2hi, in_=wh2loc[:, q, NCLASS:NCLASS + 1])
            nc.vector.tensor_tensor(out=t2hi, in0=t2l, in1=t2hi, op=SUB)
            nc.vector.tensor_copy(out=wh2loc[:, q, NCLASS + 1:NCLASS + 2], in_=t2hi)

        nc.sync.dma_start(out=ccin.ap().rearrange("(q p) c -> p q c", p=128),
                          in_=wh2loc[:, :, :])
        nc.gpsimd.collective_compute(
            "AllGather", mybir.AluOpType.bypass,
            replica_groups=[list(range(NCORES))],
            ins=[ccin.ap()], outs=[ccout.ap()])

        # t1 row broadcast: transpose t1col -> [IC, 128] -> DRAM -> bcast
        psT = ps_small.tile([IC, 128], f32, tag="psF", bufs=1)
        nc.tensor.transpose(psT, t1col, ident)
        st4 = persist.tile([IC, 128], f32, name="st4", tag="st4")
        nc.vector.tensor_copy(out=st4, in_=psT)
        dt1 = dscratch.tile([BLK], f32, tag="dt1")
        nc.sync.dma_start(out=dt1.rearrange("(a b) -> a b", a=IC), in_=st4)
        T1B = persist.tile([128, BLK], f32, name="T1B", tag="T1B")
        nc.sync.dma_start(out=T1B, in_=dbcast(dt1, 128, BLK))
        ub2 = persist.tile([128, BLK], bf16, name="ub2", tag="ub2")
        nc.scalar.activation(out=ub2, in_=T1B, func=EXP, scale=0.8)

        # per-q gathered Wh2 (+ones col) and t2-derived scalars
        wh2E_q = []
        b2c_q = []
        d2c_q = []
        ccoutQ = ccout.ap().rearrange("(k q p) c -> q p k c", p=128, q=NG)
        for q in range(NG):
            wf = persist.tile([128, GRP, NCC], bf16, name=f"wh2f{q}", tag=f"wh2f{q}")
            nc.sync.dma_start(out=wf, in_=ccoutQ[q])
            we = persist.tile([128, GRP, 65], bf16, name=f"wh2E{q}", tag=f"wh2E{q}")
            nc.vector.tensor_copy(out=we[:, :, :NCLASS], in_=wf[:, :, :NCLASS])
            nc.gpsimd.memset(we[:, :, NCLASS:64], 0.0)
            nc.gpsimd.memset(we[:, :, 64:65], 1.0)
            wh2E_q.append(we)
            t2 = persist.tile([128, GRP], f32, name=f"t2c{q}", tag=f"t2c{q}")
            nc.vector.tensor_tensor(out=t2, in0=wf[:, :, NCLASS],
                                    in1=wf[:, :, NCLASS + 1], op=ADD)
            b2 = persist.tile([128, GRP], f32, name=f"b2c{q}", tag=f"b2c{q}")
            nc.scalar.activation(out=b2, in_=t2, func=EXP)
            d2 = persist.tile([128, GRP], f32, name=f"d2c{q}", tag=f"d2c{q}")
            nc.scalar.activation(out=d2, in_=t2, func=EXP, scale=0.2)
            b2c_q.append(b2)
            d2c_q.append(d2)

        # ---------------- layer 2 attention ----------------
        psO = ps1.tile([65, BLK], f32, tag="psL", bufs=3)
        for g in range(NG):
            T2g = tp.tile([128, GRP, BLK], bf16, tag="T2g")
            for k in range(GRP):
                nc.vector.tensor_scalar(out=T2g[:, k, :], in0=ub2,
                                        scalar1=b2c_q[g][:, k:k + 1],
                                        scalar2=d2c_q[g][:, k:k + 1],
                                        op0=MULT, op1=MAX)
            Pg = tp.tile([128, GRP, BLK], bf16, tag="Pg")
            nc.gpsimd.tensor_tensor(out=Pg[:, 6:, :], in0=T2g[:, 6:, :],
                                    in1=adj_g[g][:, 6:, :], op=MULT)
            nc.vector.tensor_tensor(out=Pg[:, :6, :], in0=T2g[:, :6, :],
                                    in1=adj_g[g][:, :6, :], op=MULT)
            for k in range(GRP):
                nc.tensor.matmul(psO, lhsT=wh2E_q[g][:, k, :], rhs=Pg[:, k, :],
                                 start=(g == 0 and k == 0),
                                 stop=(g == NG - 1 and k == GRP - 1))

        # ---------------- final epilogue ----------------
        # copy [num | junk | den] to SBUF, transpose each 128-col chunk to
        # [i, 65], then normalize with the per-partition denominator, apply
        # ELU and log_softmax entirely in [i, c] layout.
        oS = persist.tile([65, BLK], f32, name="oS", tag="oS")
        nc.scalar.copy(out=oS, in_=psO)
        outR = out.rearrange("(ic p) c -> ic p c", p=128)
        for ic in range(IC):
            psF = ps_small.tile([128, 65], f32, tag="psF", bufs=1)
            nc.tensor.transpose(psF, oS[:, ic * 128:(ic + 1) * 128], ident[:65, :65])
            rc = ep.tile([128, 1], f32, tag="rc")
            nc.vector.reciprocal(out=rc, in_=psF[:, 64:65])
            xv = ep.tile([128, NCLASS], f32, tag="xv")
            nc.vector.tensor_scalar(out=xv, in0=psF[:, :NCLASS], scalar1=rc,
                                    scalar2=None, op0=MULT)
            # elu(x) = relu(x) - relu(1 - e^x)
            ev = ep.tile([128, NCLASS], f32, tag="ev")
            nc.scalar.activation(out=ev, in_=xv, func=EXP)
            nc.scalar.activation(out=ev, in_=ev, func=RELU, scale=-1.0, bias=1.0)
            rv = ep.tile([128, NCLASS], f32, tag="rv")
            nc.scalar.activation(out=rv, in_=xv, func=RELU)
            el = ep.tile([128, NCLASS], f32, tag="el")
            nc.vector.tensor_tensor(out=el, in0=rv, in1=ev, op=SUB)
            # log_softmax along classes
            mx = ep.tile([128, 1], f32, tag="mx")
            nc.vector.tensor_reduce(out=mx, in_=el, axis=AX, op=MAX)
            sh = ep.tile([128, NCLASS], f32, tag="sh")
            nc.vector.tensor_scalar(out=sh, in0=el, scalar1=mx, scalar2=None,
                                    op0=SUB)
            esum = ep.tile([128, 1], f32, tag="esum")
            et = ep.tile([128, NCLASS], f32, tag="et")
            nc.scalar.activation(out=et, in_=sh, func=EXP, accum_out=esum)
            lse = ep.tile([128, 1], f32, tag="lse")
            nc.scalar.activation(out=lse, in_=esum, func=LN)
            of = ep.tile([128, NCLASS], f32, tag="of")
            nc.vector.tensor_scalar(out=of, in0=sh, scalar1=lse, scalar2=None,
                                    op0=SUB)
            nc.sync.dma_start(out=outR[ic], in_=of)

    nc.compile()
    return nc


def _host_precompute(x, adj, W_heads, a_heads, W_out, a_out):
    x = np.asarray(x, np.float32)
    adj = np.asarray(adj, np.float32)
    W_heads = np.asarray(W_heads, np.float32)
    a_heads = np.asarray(a_heads, np.float32)
    W_out = np.asarray(W_out, np.float32)
    a_out = np.asarray(a_out, np.float32)

    # Wh[h] = x @ W_heads[h]  -> [H, N, D]
    Wh = np.einsum("nf,hfd->hnd", x, W_heads, optimize=True).astype(np.float32)
    a1 = a_heads[:, :NHID, 0]
    a2 = a_heads[:, NHID:, 0]
    s1 = np.einsum("hnd,hd->hn", Wh, a1)          # [H, N]
    s2 = np.einsum("hnd,hd->hn", Wh, a2)          # [H, N]

    whe = np.empty((N, NHEADS, 65), np.float32)
    whe[:, :, :64] = np.transpose(Wh, (1, 0, 2))
    whe[:, :, 64] = 1.0
    # row jc*128+p -> [q, p, k, :] with jc = 4k + q (strided j-groups)
    whe = np.ascontiguousarray(
        whe.reshape(GRP, NG, 128, NHEADS * 65).transpose(1, 2, 0, 3)).astype(BF16)

    sdb = np.empty((N, 24), np.float32)
    sdb[:, 0:8] = s2.T
    sdb[:, 8:16] = np.exp(0.2 * s2.T)
    sdb[:, 16:24] = np.exp(s2.T)
    # row jc*128+p -> [p, jc, :]
    sdb = np.ascontiguousarray(
        sdb.reshape(JC, 128, 24).transpose(1, 0, 2)).reshape(128, JC * 24)

    a1o = np.ascontiguousarray(a_out[:NCLASS, 0], dtype=np.float32)
    a2o32 = np.tile(np.ascontiguousarray(a_out[NCLASS:, 0]), JC).astype(np.float32)
    woutb = W_out.astype(BF16)

    in_maps = []
    for c in range(NCORES):
        sl = slice(c * BLK, (c + 1) * BLK)
        in_maps.append({
            "whe": whe,
            "adjt": np.ascontiguousarray(
                adj[sl, :].T.reshape(GRP, NG, 128, BLK).transpose(1, 2, 0, 3)
            ).astype(BF16),
            "ublk": np.exp(0.8 * s1[:, sl]).astype(BF16),
            "sdb": sdb,
            "wout": woutb,
            "a1o": a1o,
            "a2o32": a2o32,
        })
    return in_maps


def kernel(x, adj, W_heads, a_heads, W_out, a_out):
    global LAST_RESULTS
    from concourse.bass_utils import run_bass_kernel_spmd

    if "nc" not in _CACHE:
        _CACHE["nc"] = _build_program()
    nc = _CACHE["nc"]

    in_maps = _host_precompute(x, adj, W_heads, a_heads, W_out, a_out)
    res = run_bass_kernel_spmd(nc, in_maps, core_ids=list(range(NCORES)))
    LAST_RESULTS = res
    return np.concatenate([res.results[c]["out"] for c in range(NCORES)], axis=0)
